# revision 1
# baseline (speedup 1.0000x reference)
"""GAT message-passing kernel for TRN2: host preprocessing + Bass/Tile program builder.

Design (per core, SPMD over 8 cores, nodes sharded by destination block):
  phase 0: feat = x @ W_gat for own node shard (bf16), er = feat . attn_r;
           AllGather bf16 feat table [n_nodes, hid] to every core's DRAM.
  edge phase: edges pre-sorted by (dst block, src). Per dst block of 128 nodes:
      - dma_gather feat rows of the block's edge sources (bf16, 256B rows),
        in two gathers (low/high half-table) to satisfy int16 indices.
      - per 128-edge chunk: el = rowsum(G * attn_l)  (DVE accum),
        er = rowsum(onehot(dst) * er_bcast) (DVE accum),
        alpha = exp(leakyrelu_0.2(el+er)) (ACT, batched per block),
        O_alpha = is_equal(iota, dst) * alpha (one fused DVE op, bf16),
        PSUM_A += O_alpha^T @ G      (agg unnorm, [128 dst, hid])
        PSUM_B += ones^T @ O_alpha   (denom,      [1, 128 dst])
      - tail: recip(max(denom,eps)) -> per-dst scale; gene = leaky(agg*scale + bias);
        out_block = gene @ W_lin^T via PE transpose + fp32 matmul.
Softmax max-subtraction is dropped (exp args bounded ~ +-8; ratios identical).
"""

import numpy as np
import ml_dtypes
from contextlib import ExitStack

import concourse.bass as bass
import concourse.tile as tile
from concourse import bacc, mybir
from concourse.masks import make_identity
from concourse import library_config

dt = mybir.dt
P = 128
PAD_DST = 512.0  # one-hot miss sentinel (exact in bf16, > 127)


# ---------------------------------------------------------------- host side

def preprocess(src, dst, n_nodes, n_cores):
    """Pure index-space preprocessing (no float math on values).

    Returns (schedule, per_core) where schedule has the common per-block chunk
    counts and per_core[c] has idx16 / dstf arrays.
    """
    src = np.asarray(src).astype(np.int64)
    dst = np.asarray(dst).astype(np.int64)
    npc = n_nodes // n_cores                      # nodes per core
    assert npc * n_cores == n_nodes
    blocks = (npc + P - 1) // P
    npc_pad = blocks * P                          # padded nodes per core
    n_pad = npc_pad * n_cores                     # padded global node count
    half = n_pad // 2                             # low table rows [0, half)
    assert half <= 32767 and (n_pad - half) <= 32767
    assert half % npc_pad == 0                    # half boundary between cores

    core_of = dst // npc
    blk_of = (dst % npc) // P
    dloc_of = (dst % npc) % P
    src = (src // npc) * npc_pad + (src % npc)    # padded source coordinates

    # bucket edges: lists[(core, block, is_hi)] -> (src_idx_in_half, dst_local)
    lo_lists = [[[] for _ in range(blocks)] for _ in range(n_cores)]
    hi_lists = [[[] for _ in range(blocks)] for _ in range(n_cores)]
    order = np.lexsort((src, blk_of, core_of))
    s_s, c_s, b_s, d_s = src[order], core_of[order], blk_of[order], dloc_of[order]
    hi_mask = s_s >= half
    for c in range(n_cores):
        cm = c_s == c
        for b in range(blocks):
            m = cm & (b_s == b)
            ml = m & ~hi_mask
            mh = m & hi_mask
            lo_lists[c][b] = (s_s[ml], d_s[ml])
            hi_lists[c][b] = (s_s[mh] - half, d_s[mh])

    def nchunks(n):
        return (n + P - 1) // P

    C_lo = [max(max(nchunks(len(lo_lists[c][b][0])) for c in range(n_cores)), 1)
            for b in range(blocks)]
    C_hi = [max(nchunks(len(hi_lists[c][b][0])) for c in range(n_cores))
            for b in range(blocks)]

    total_chunks = sum(C_lo) + sum(C_hi)
    total_L = total_chunks * P

    per_core = []
    for c in range(n_cores):
        idx = np.zeros(total_L, dtype=np.int16)
        dstf = np.full(total_L, PAD_DST, dtype=np.float32)
        off = 0
        for b in range(blocks):
            for lists, C in ((lo_lists, C_lo[b]), (hi_lists, C_hi[b])):
                L = C * P
                if L == 0:
                    continue
                s_arr, d_arr = lists[c][b]
                n = len(s_arr)
                idx[off:off + n] = s_arr.astype(np.int16)
                # pads keep idx 0 (real row; killed by dst sentinel)
                dstf[off:off + n] = d_arr.astype(np.float32)
                off += L
        assert off == total_L
        # wrapped int16 layout: index i lives at [i % 16, i // 16];
        # replicated 8x down partitions (one copy per Q7 core)
        idx16 = np.tile(idx.reshape(total_L // 16, 16).T, (8, 1)).copy()
        # chunk-major dst columns: chunk g partition e = dstf[g*128+e]
        dstf2 = dstf.reshape(total_chunks, P).T.copy()
        per_core.append({"idx16": idx16, "dstf": dstf2})

    sched = {
        "n_nodes": n_nodes, "n_cores": n_cores, "npc": npc, "blocks": blocks,
        "npc_pad": npc_pad, "n_pad": n_pad,
        "half": half, "C_lo": C_lo, "C_hi": C_hi,
        "total_chunks": total_chunks, "total_L": total_L,
    }
    return sched, per_core


def make_core_inputs(sched, per_core, x, W_gat, attn_l, attn_r, bias_gat, W_lin):
    """Build per-core in_maps. Only permutation/replication/padding of values."""
    n_cores, npc, blocks = sched["n_cores"], sched["npc"], sched["blocks"]
    in_f = x.shape[1]
    hid = W_gat.shape[1]
    x = np.asarray(x, dtype=np.float32)
    in_maps = []
    for c in range(n_cores):
        xs = x[c * npc:(c + 1) * npc]                      # [npc, in_f]
        xpad = np.zeros((blocks * P, in_f), dtype=np.float32)
        xpad[:npc] = xs
        m = {
            "xT": np.ascontiguousarray(xpad.T),            # [in_f, blocks*P]
            "Wg": np.asarray(W_gat, dtype=np.float32),
            "attnl_b": np.broadcast_to(np.asarray(attn_l, np.float32), (P, hid)).copy(),
            "attnr_b": np.broadcast_to(np.asarray(attn_r, np.float32), (P, hid)).copy(),
            "bias_b": np.broadcast_to(np.asarray(bias_gat, np.float32), (P, hid)).copy(),
            "WlT": np.ascontiguousarray(np.asarray(W_lin, np.float32).T),  # [hid, out_f]
            "iota_bf": np.broadcast_to(
                np.arange(P, dtype=ml_dtypes.bfloat16), (P, P)).copy(),
            "iota_f": np.broadcast_to(
                np.arange(P, dtype=np.float32), (P, P)).copy(),
            "ident": np.eye(P, dtype=np.float32),
            "idx16": per_core[c]["idx16"],
            "dstf": per_core[c]["dstf"],
        }
        in_maps.append(m)
    return in_maps


# ---------------------------------------------------------------- device side

def build_program(sched, in_f, hid, out_f, attn_slope=0.2, act_slope=0.01,
                  n_repeat=1):
    n_cores = sched["n_cores"]
    npc, blocks, half = sched["npc"], sched["blocks"], sched["half"]
    C_lo, C_hi = sched["C_lo"], sched["C_hi"]
    total_chunks, total_L = sched["total_chunks"], sched["total_L"]
    assert in_f % P == 0 and hid == P
    KT = in_f // P

    nc = bacc.Bacc("TRN2", target_bir_lowering=False, debug=False,
                   num_devices=n_cores)

    def din(name, shape, dtype):
        return nc.dram_tensor(name, shape, dtype, kind="ExternalInput").ap()

    xT = din("xT", [in_f, blocks * P], dt.float32)
    Wg = din("Wg", [in_f, hid], dt.float32)
    attnl_b = din("attnl_b", [P, hid], dt.float32)
    attnr_b = din("attnr_b", [P, hid], dt.float32)
    bias_b = din("bias_b", [P, hid], dt.float32)
    WlT = din("WlT", [hid, out_f], dt.float32)
    iota_bf = din("iota_bf", [P, P], dt.bfloat16)
    iota_f = din("iota_f", [P, P], dt.float32)
    ident_in = din("ident", [P, P], dt.float32)
    idx16 = din("idx16", [128, total_L // 16], dt.int16)
    dstf = din("dstf", [P, total_chunks], dt.float32)
    out = nc.dram_tensor("out", [blocks * P, out_f], dt.float32,
                         kind="ExternalOutput").ap()

    tableShard = nc.dram_tensor("tableShard", [blocks * P, hid],
                               dt.bfloat16).ap()
    tableFull = nc.dram_tensor("tableFull", [sched["n_pad"], hid],
                               dt.bfloat16, addr_space="Shared").ap()
    er_lin = nc.dram_tensor("er_lin", [blocks * P], dt.float32).ap()

    with ExitStack() as ctx:
        tc = ctx.enter_context(tile.TileContext(nc))
        nc.gpsimd.load_library(library_config.mlp)
        const = ctx.enter_context(tc.tile_pool(name="const", bufs=1))

        # ---- constants in SBUF
        iota_bf_sb = const.tile([P, P], dt.bfloat16)
        nc.sync.dma_start(iota_bf_sb[:], iota_bf[:])
        iota_f_sb = const.tile([P, P], dt.float32)
        nc.sync.dma_start(iota_f_sb[:], iota_f[:])
        attnl_sb = const.tile([P, hid], dt.float32)
        nc.sync.dma_start(attnl_sb[:], attnl_b[:])
        attnr_sb = const.tile([P, hid], dt.float32)
        nc.sync.dma_start(attnr_sb[:], attnr_b[:])
        bias_sb = const.tile([P, hid], dt.float32)
        nc.sync.dma_start(bias_sb[:], bias_b[:])
        WlT_sb = const.tile([hid, out_f], dt.float32)
        nc.sync.dma_start(WlT_sb[:], WlT[:])
        ident_sb = const.tile([P, P], dt.float32)
        nc.sync.dma_start(ident_sb[:], ident_in[:])
        ones_col_bf = const.tile([P, 1], dt.bfloat16)
        nc.vector.memset(ones_col_bf[:], 1.0)
        ones_row_f = const.tile([1, P], dt.float32)
        nc.vector.memset(ones_row_f[:], 1.0)
        one11_f = const.tile([1, 1], dt.float32)
        nc.vector.memset(one11_f[:], 1.0)
        idx_sb = const.tile([128, total_L // 16], dt.int16)
        nc.sync.dma_start(idx_sb[:], idx16[:])
        dst_sb = const.tile([P, total_chunks], dt.float32)
        nc.sync.dma_start(dst_sb[:], dstf[:])

        # ---- phase 0: feat shard + er + table AllGather
        ph = ctx.enter_context(tc.tile_pool(name="ph", bufs=1))
        xT_sb = []
        Wg_sb = []
        for k in range(KT):
            tf = ph.tile([P, blocks * P], dt.float32, tag="xTf")
            nc.sync.dma_start(tf[:], xT[k * P:(k + 1) * P, :])
            t = ph.tile([P, blocks * P], dt.bfloat16, tag=f"xT{k}")
            nc.vector.tensor_copy(t[:], tf[:])
            xT_sb.append(t)
            wf = ph.tile([P, hid], dt.float32, tag="Wgf")
            nc.sync.dma_start(wf[:], Wg[k * P:(k + 1) * P, :])
            w = ph.tile([P, hid], dt.bfloat16, tag=f"Wg{k}")
            nc.vector.tensor_copy(w[:], wf[:])
            Wg_sb.append(w)
        er_sb = const.tile([P, blocks], dt.float32)

        psA = ctx.enter_context(tc.tile_pool(name="psA", bufs=2, space="PSUM"))
        psB = ctx.enter_context(tc.tile_pool(name="psB", bufs=2, space="PSUM"))
        psErb = ctx.enter_context(tc.tile_pool(name="psErb", bufs=2, space="PSUM"))
        psTail = ctx.enter_context(tc.tile_pool(name="psTail", bufs=2, space="PSUM"))

        featp = ctx.enter_context(tc.tile_pool(name="featp", bufs=3))
        scrp = ctx.enter_context(tc.tile_pool(name="scrp", bufs=3))

        for nb in range(blocks):
            fp = psA.tile([P, hid], dt.float32, tag="psA")
            for k in range(KT):
                nc.tensor.matmul(fp[:], lhsT=xT_sb[k][:, nb * P:(nb + 1) * P],
                                 rhs=Wg_sb[k][:], start=(k == 0),
                                 stop=(k == KT - 1))
            fbf = featp.tile([P, hid], dt.bfloat16, tag="feat")
            nc.vector.tensor_copy(fbf[:], fp[:])
            scr = scrp.tile([P, hid], dt.float32, tag="scr")
            nc.vector.scalar_tensor_tensor(
                out=scr[:], in0=fp[:], scalar=1.0, in1=attnr_sb[:],
                op0=mybir.AluOpType.bypass, op1=mybir.AluOpType.mult,
                accum_out=er_sb[:, nb:nb + 1])
            nc.sync.dma_start(tableShard[nb * P:(nb + 1) * P, :], fbf[:])
            nc.sync.dma_start(er_lin[nb * P:(nb + 1) * P, None],
                              er_sb[:, nb:nb + 1])

        nc.gpsimd.collective_compute(
            "AllGather", mybir.AluOpType.bypass,
            replica_groups=[list(range(n_cores))],
            ins=[tableShard[:].opt()], outs=[tableFull[:].opt()])

        # ---- edge phase
        gp = ctx.enter_context(tc.tile_pool(name="gp", bufs=2))
        oer = ctx.enter_context(tc.tile_pool(name="oer", bufs=3))
        gel = ctx.enter_context(tc.tile_pool(name="gel", bufs=3))
        oap = ctx.enter_context(tc.tile_pool(name="oap", bufs=3))
        sp = ctx.enter_context(tc.tile_pool(name="sp", bufs=2))
        tp = ctx.enter_context(tc.tile_pool(name="tp", bufs=2))

        loop_ctx = tc.For_i(0, n_repeat, 1) if n_repeat > 1 else None
        if loop_ctx is not None:
            loop_ctx.__enter__()
        if True:
          g = 0
          for b in range(blocks):
            Cl, Ch = C_lo[b], C_hi[b]
            C = Cl + Ch
            # er row -> [P,P] broadcast via rank-1 matmul (stays in PSUM)
            er_row = tp.tile([1, P], dt.float32, tag="er_row")
            nc.sync.dma_start(er_row[:], er_lin[b * P:(b + 1) * P][None, :])
            erb_ps = psErb.tile([P, P], dt.float32, tag="erb")
            nc.tensor.matmul(erb_ps[:], lhsT=ones_row_f[:], rhs=er_row[:],
                             start=True, stop=True)

            G = gp.tile([P, C * hid], dt.bfloat16, tag="G")
            G3 = G[:].rearrange("p (c h) -> p c h", h=hid)
            o16 = (g * P) // 16
            GMAX = 6  # chunks per dma_gather; 768 idxs < 1024-desc SWDGE ring
            for cbase, ccnt, tbl in (
                    [(c0, min(GMAX, Cl - c0), tableFull[0:half, :])
                     for c0 in range(0, Cl, GMAX)] +
                    [(Cl + c0, min(GMAX, Ch - c0),
                      tableFull[half:sched["n_pad"], :])
                     for c0 in range(0, Ch, GMAX)]):
                nc.gpsimd.dma_gather(
                    G3[:, cbase:cbase + ccnt, :], tbl,
                    idx_sb[:, o16 + cbase * 8:o16 + (cbase + ccnt) * 8],
                    ccnt * P, ccnt * P, hid, elem_step=hid)

            el_all = sp.tile([P, C], dt.float32, tag="el")
            er_all = sp.tile([P, C], dt.float32, tag="er")
            for j in range(C):
                gj = g + j
                o1 = oer.tile([P, P], dt.float32, tag="oer")
                nc.vector.scalar_tensor_tensor(
                    out=o1[:], in0=iota_f_sb[:], scalar=dst_sb[:, gj:gj + 1],
                    in1=erb_ps[:], op0=mybir.AluOpType.is_equal,
                    op1=mybir.AluOpType.mult,
                    accum_out=er_all[:, j:j + 1])
                o2 = gel.tile([P, hid], dt.float32, tag="gel")
                nc.vector.scalar_tensor_tensor(
                    out=o2[:], in0=G3[:, j, :], scalar=1.0, in1=attnl_sb[:],
                    op0=mybir.AluOpType.bypass, op1=mybir.AluOpType.mult,
                    accum_out=el_all[:, j:j + 1])
            z = sp.tile([P, C], dt.float32, tag="z")
            nc.vector.tensor_tensor(out=z[:], in0=el_all[:], in1=er_all[:],
                                    op=mybir.AluOpType.add)
            lz = sp.tile([P, C], dt.float32, tag="lz")
            nc.vector.scalar_tensor_tensor(
                out=lz[:], in0=z[:], scalar=float(attn_slope), in1=z[:],
                op0=mybir.AluOpType.mult, op1=mybir.AluOpType.max)
            alpha = sp.tile([P, C], dt.float32, tag="alpha")
            nc.scalar.activation(alpha[:], lz[:],
                                 mybir.ActivationFunctionType.Exp)

            pA = psA.tile([P, hid], dt.float32, tag="psA")
            pB = psB.tile([1, P], dt.float32, tag="psB")
            for j in range(C):
                gj = g + j
                Oa = oap.tile([P, P], dt.bfloat16, tag="oa")
                nc.vector.tensor_scalar(
                    Oa[:], iota_bf_sb[:], dst_sb[:, gj:gj + 1],
                    alpha[:, j:j + 1], mybir.AluOpType.is_equal,
                    mybir.AluOpType.mult)
                nc.tensor.matmul(pA[:], lhsT=Oa[:], rhs=G3[:, j, :],
                                 start=(j == 0), stop=(j == C - 1))
                nc.tensor.matmul(pB[:], lhsT=ones_col_bf[:], rhs=Oa[:],
                                 start=(j == 0), stop=(j == C - 1))
            g += C

            den = tp.tile([1, P], dt.float32, tag="den")
            nc.vector.tensor_scalar(den[:], pB[:], 1e-30, None,
                                    mybir.AluOpType.max)
            rec = tp.tile([1, P], dt.float32, tag="rec")
            nc.vector.reciprocal(rec[:], den[:])
            rcol_ps = psTail.tile([P, 1], dt.float32, tag="rcol")
            nc.tensor.matmul(rcol_ps[:], lhsT=rec[:], rhs=one11_f[:],
                             start=True, stop=True)
            gene = tp.tile([P, hid], dt.float32, tag="gene")
            nc.vector.scalar_tensor_tensor(
                out=gene[:], in0=pA[:], scalar=rcol_ps[:, 0:1], in1=bias_sb[:],
                op0=mybir.AluOpType.mult, op1=mybir.AluOpType.add)
            geneL = tp.tile([P, hid], dt.float32, tag="geneL")
            nc.vector.scalar_tensor_tensor(
                out=geneL[:], in0=gene[:], scalar=float(act_slope),
                in1=gene[:], op0=mybir.AluOpType.mult, op1=mybir.AluOpType.max)
            gT_ps = psErb.tile([hid, P], dt.float32, tag="erb")
            nc.tensor.transpose(gT_ps[:], geneL[:], ident_sb[:])
            gT = tp.tile([hid, P], dt.float32, tag="gT")
            nc.vector.tensor_copy(gT[:], gT_ps[:])
            o_ps = psTail.tile([P, out_f], dt.float32, tag="rcol")
            nc.tensor.matmul(o_ps[:], lhsT=gT[:], rhs=WlT_sb[:],
                             start=True, stop=True)
            o_sb = tp.tile([P, out_f], dt.float32, tag="osb")
            nc.vector.tensor_copy(o_sb[:], o_ps[:])
            nc.sync.dma_start(out[b * P:(b + 1) * P, :], o_sb[:])
        if loop_ctx is not None:
            loop_ctx.__exit__(None, None, None)

    nc.compile()
    return nc




# ---------------------------------------------------------------- entry point

N_NODES, N_EDGES, IN_F, HID, OUT_F = 50000, 800000, 256, 128, 64
N_CORES = 8

_cache = {}


def kernel(x, src, dst, W_gat, attn_l, attn_r, bias_gat, W_lin):
    """Full-input GAT layer on 8 NeuronCores; returns [N_NODES, OUT_F] fp32."""
    from concourse.bass_utils import run_bass_kernel_spmd

    src = np.asarray(src)
    dst = np.asarray(dst)
    key = (src.tobytes(), dst.tobytes())
    ck = _cache.get("k")
    if ck is not None and ck[0] == key:
        sched, nc = ck[1], ck[2]
    else:
        sched, per_core = preprocess(src, dst, N_NODES, N_CORES)
        _cache["pc"] = per_core
        nc = build_program(sched, IN_F, HID, OUT_F)
        _cache["k"] = (key, sched, nc)
        ck = _cache["k"]
    sched = ck[1]
    per_core = _cache["pc"]
    in_maps = make_core_inputs(sched, per_core, x, W_gat, attn_l, attn_r,
                               bias_gat, W_lin)
    res = run_bass_kernel_spmd(nc, in_maps, core_ids=list(range(N_CORES)))
    out = np.concatenate(
        [res.results[c]["out"][:sched["npc"]] for c in range(N_CORES)], axis=0)
    return out.astype(np.float32)

